# revision 5
# baseline (speedup 1.0000x reference)
"""TRN2 Bass kernel for single-head causal attention (nn_Head).

Reference math (note k/q role swap vs standard attention):
    k = x @ Wk; q = x @ Wq; v = x @ Wv          # [B, T, H]
    S = (k @ q^T) * C**-0.5                     # [B, T, T], rows = k positions
    S = where(tril, S, -inf); P = softmax(S, axis=-1)
    out = P @ v                                 # [B, T, H]

Strategy (8 NeuronCores, data-parallel over batch, 4 batches/core):
  - Host marshalling: upload x pre-transposed (xT [b, C, T]) so the
    contraction dim C lands on SBUF partitions; fold the 1/sqrt(C) scale
    into Wk; concatenate Wk|Wq so one matmul computes both projections.
  - All matmuls in float32r (TF32-class, full PE rate at free dim >= 256).
  - S^T tiles [b_part, a_free] (b = softmax dim): no max subtraction
    needed (|S| <= ~1.5 by construction), so P = exp(S) and the
    denominator is folded into the PV matmul via a ones-column on V.
  - PV in transposed form: out'^T[h', a] = sum_b V'[b, h'] P^T[b, a]
    accumulated over b-tiles in PSUM; final PE transpose + DVE divide.
  - Batch-level software pipelining: batch b+1's projection matmuls are
    interleaved into batch b's attention phase so TensorE always has
    dense independent work (keeps the HAM clock gate at K=8/8).
"""

import sys

sys.path.insert(0, "/opt/trn_rl_repo")

import numpy as np

import concourse.bass as bass  # noqa: F401
import concourse.mybir as mybir
import concourse.tile as tile
from concourse import bacc
from concourse.bass_utils import run_bass_kernel_spmd
from concourse.masks import make_identity

B, T, C, H = 32, 2048, 1024, 64
N_CORES = 8
BPC = B // N_CORES  # batches per core
SCALE = C**-0.5

F32 = mybir.dt.float32
F32R = mybir.dt.float32r

TCH = 512  # i-chunk width (PSUM bank = 512 fp32)
NTC = T // TCH  # 4 i-chunks
NJT = T // 128  # 16 j-tiles
NCC = C // 128  # 8 contraction chunks
PV_LAG = 3  # S^T(jt) to PV(jt) interleave distance


def build_program():
    nc = bacc.Bacc(
        "TRN2", target_bir_lowering=False, debug=False, num_devices=N_CORES
    )
    xT = nc.dram_tensor("xT", [BPC, C, T], F32R, kind="ExternalInput").ap()
    Wkq = nc.dram_tensor("Wkq", [C, 128], F32R, kind="ExternalInput").ap()
    Wv = nc.dram_tensor("Wv", [C, H], F32R, kind="ExternalInput").ap()
    out = nc.dram_tensor("out", [BPC, T, H], F32, kind="ExternalOutput").ap()

    with tile.TileContext(nc) as tc:
        with (
            tc.tile_pool(name="const", bufs=1) as const,
            tc.tile_pool(name="xtp", bufs=11) as xtp,
            tc.tile_pool(name="ep", bufs=2) as ep,
            tc.tile_pool(name="ptp", bufs=6) as ptp,
            tc.tile_pool(name="psA", bufs=2, space="PSUM") as psA,
            tc.tile_pool(name="psS", bufs=3, space="PSUM") as psS,
            tc.tile_pool(name="psO", bufs=2, space="PSUM") as psO,
            tc.tile_pool(name="psT", bufs=1, space="PSUM") as psT,
        ):
            # --- constants ---
            wkq_sb = const.tile([128, NCC, 128], F32R)
            nc.sync.dma_start(
                out=wkq_sb, in_=Wkq.rearrange("(cc p) h -> p cc h", p=128)
            )
            wv_sb = const.tile([128, NCC, H], F32R)
            nc.sync.dma_start(
                out=wv_sb, in_=Wv.rearrange("(cc p) h -> p cc h", p=128)
            )
            ident = const.tile([128, 128], F32)
            make_identity(nc, ident)
            # causal tile mask: trimask[j, i] = 1.0 if i >= j else 0.0
            # (built in fp32; DVE copy rounds into the fp32r tile)
            trimask_f = const.tile([128, 128], F32)
            nc.vector.memset(trimask_f, 1.0)
            nc.gpsimd.affine_select(
                out=trimask_f,
                in_=trimask_f,
                compare_op=mybir.AluOpType.is_ge,
                fill=0.0,
                base=0,
                pattern=[[1, 128]],
                channel_multiplier=-1,
            )
            trimask = const.tile([128, 128], F32R)
            nc.vector.tensor_copy(trimask, trimask_f)
            ones_f = const.tile([128, 1], F32)
            nc.vector.memset(ones_f, 1.0)

            state = {}

            def emit_loads(b):
                xts = []
                for cc in range(NCC):
                    xt = xtp.tile([128, T], F32R, name=f"xt_{b}_{cc}", tag="xt")
                    # alternate HWDGE queues (sync/scalar) for 2-way overlap
                    eng = nc.sync if cc % 2 == 0 else nc.scalar
                    eng.dma_start(out=xt, in_=xT[b, cc * 128 : (cc + 1) * 128, :])
                    xts.append(xt)
                state[b] = {"xts": xts}

            def emit_kq_chunk(b, t_):
                st = state[b]
                if "kqT" not in st:
                    st["kqT"] = ep.tile([128, T], F32R, name=f"kqT_{b}", tag="kqT")
                    st["qT"] = ep.tile([64, T], F32R, name=f"qT_{b}", tag="qT")
                sl = slice(t_ * TCH, (t_ + 1) * TCH)
                kq_ps = psA.tile([128, TCH], F32, name=f"kqps_{b}_{t_}", tag="proj")
                for cc in range(NCC):
                    nc.tensor.matmul(
                        kq_ps,
                        wkq_sb[:, cc, :],
                        st["xts"][cc][:, sl],
                        start=(cc == 0),
                        stop=(cc == NCC - 1),
                    )
                nc.vector.tensor_copy(st["kqT"][:, sl], kq_ps)
                # shift q^T half down to partitions 0-63 (SWDGE: keep the
                # sync/scalar HWDGE queues free for bulk loads)
                nc.gpsimd.dma_start(out=st["qT"][:, sl], in_=st["kqT"][64:128, sl])

            def emit_v_chunk(b, t_):
                st = state[b]
                if "vT" not in st:
                    st["vT"] = ep.tile([64, T], F32, name=f"vT_{b}", tag="vT")
                sl = slice(t_ * TCH, (t_ + 1) * TCH)
                v_ps = psA.tile([64, TCH], F32, name=f"vps_{b}_{t_}", tag="proj")
                for cc in range(NCC):
                    nc.tensor.matmul(
                        v_ps,
                        wv_sb[:, cc, :],
                        st["xts"][cc][:, sl],
                        start=(cc == 0),
                        stop=(cc == NCC - 1),
                    )
                nc.vector.tensor_copy(st["vT"][:, sl], v_ps)

            def emit_vtrans(b, jt):
                st = state[b]
                if "vp" not in st:
                    st["vp"] = ep.tile(
                        [128, NJT, H + 1], F32R, name=f"vp_{b}", tag="vp"
                    )
                tv_ps = psT.tile([128, H + 1], F32, name=f"tvps_{b}_{jt}", tag="tps")
                nc.tensor.transpose(
                    tv_ps[:, 0:H],
                    st["vT"][:, jt * 128 : (jt + 1) * 128],
                    ident[0:64, 0:64],
                )
                nc.vector.tensor_copy(st["vp"][:, jt, 0:H], tv_ps[:, 0:H])
                nc.vector.tensor_copy(st["vp"][:, jt, H : H + 1], ones_f)

            def proj_units(b):
                """Work units of batch b's projection phase, for interleaving."""
                yield lambda: emit_kq_chunk(b, 0)
                yield lambda: emit_kq_chunk(b, 1)
                yield lambda: emit_kq_chunk(b, 2)
                yield lambda: emit_kq_chunk(b, 3)
                yield lambda: emit_v_chunk(b, 0)
                yield lambda: emit_v_chunk(b, 1)
                yield lambda: emit_v_chunk(b, 2)
                yield lambda: emit_v_chunk(b, 3)
                for jt0 in range(0, NJT, 4):
                    def vt_group(jt0=jt0):
                        for jt in range(jt0, jt0 + 4):
                            emit_vtrans(b, jt)
                    yield vt_group

            def emit_attention_ci(b, ci):
                st = state[b]
                kqT, qT, vp = st["kqT"], st["qT"], st["vp"]
                jt_max = 4 * ci + 3
                pts = []

                def pv_step(jt):
                    pt, i0, w = pts[jt]
                    nc.tensor.matmul(
                        o_ps[:, i0 - ci * TCH : TCH],
                        vp[:, jt, :],
                        pt[:, 0:w],
                        start=(jt == 0),
                        stop=(jt == jt_max),
                    )

                o_ps = psO.tile([H + 1, TCH], F32, name=f"ops_{b}_{ci}", tag="ops")
                for jt in range(jt_max + 1):
                    i0 = max(ci * TCH, jt * 128)
                    w = (ci + 1) * TCH - i0
                    s_ps = psS.tile(
                        [128, TCH], F32, name=f"sps_{b}_{ci}_{jt}", tag="sps"
                    )
                    # s_ps[b_local, a] = sum_h q[jt*128+b_local, h] k[a, h]
                    nc.tensor.matmul(
                        s_ps[:, 0:w],
                        qT[:, jt * 128 : (jt + 1) * 128],
                        kqT[0:64, i0 : i0 + w],
                        start=True,
                        stop=True,
                    )
                    pt = ptp.tile([128, TCH], F32R, name=f"pt_{b}_{ci}_{jt}", tag="pt")
                    nc.scalar.activation(
                        pt[:, 0:w], s_ps[:, 0:w], mybir.ActivationFunctionType.Exp
                    )
                    if jt >= 4 * ci:
                        nc.vector.tensor_mul(pt[:, 0:128], pt[:, 0:128], trimask)
                    pts.append((pt, i0, w))
                    if jt >= PV_LAG:
                        pv_step(jt - PV_LAG)
                for jt in range(max(0, jt_max + 1 - PV_LAG), jt_max + 1):
                    pv_step(jt)

                # --- epilogue: transpose out'^T, divide by denominator ---
                o_sb = ep.tile([H + 1, TCH], F32, name=f"osb_{b}_{ci}", tag="osb")
                nc.vector.tensor_copy(o_sb, o_ps)
                out_sb = ep.tile([128, 4, H], F32, name=f"outsb_{b}_{ci}", tag="outsb")
                for it in range(4):
                    to_ps = psT.tile(
                        [128, H + 1], F32, name=f"tops_{b}_{ci}_{it}", tag="tps"
                    )
                    nc.tensor.transpose(
                        to_ps,
                        o_sb[:, it * 128 : (it + 1) * 128],
                        ident[0 : H + 1, 0 : H + 1],
                    )
                    r_sb = ep.tile([128, 1], F32, name=f"rsb_{b}_{ci}_{it}", tag="rsb")
                    nc.vector.reciprocal(r_sb, to_ps[:, H : H + 1])
                    nc.vector.tensor_scalar_mul(out_sb[:, it, :], to_ps[:, 0:H], r_sb)
                nc.gpsimd.dma_start(
                    out=out[b, ci * TCH : (ci + 1) * TCH, :].rearrange(
                        "(it p) h -> p it h", p=128
                    ),
                    in_=out_sb,
                )

            # --- schedule: batch-level software pipeline ---
            emit_loads(0)
            for u in proj_units(0):
                u()
            # filler distribution: after each attention ci-group of batch b,
            # emit some of batch b+1's projection work
            fill_after_ci = [2, 3, 3, 4]  # sums to 12 units
            for b in range(BPC):
                if b + 1 < BPC:
                    emit_loads(b + 1)
                    filler = list(proj_units(b + 1))
                else:
                    filler = []
                fi = 0
                for ci in range(NTC):
                    emit_attention_ci(b, ci)
                    for _ in range(fill_after_ci[ci]):
                        if fi < len(filler):
                            filler[fi]()
                            fi += 1
                while fi < len(filler):
                    filler[fi]()
                    fi += 1

    nc.compile()
    return nc


_CACHE = {}


def _get_program():
    if "nc" not in _CACHE:
        _CACHE["nc"] = build_program()
    return _CACHE["nc"]


def kernel(x, Wk, Wq, Wv, _trace=False, _trace_kwargs=None):
    x = np.asarray(x, dtype=np.float32)
    Wk = np.asarray(Wk, dtype=np.float32)
    Wq = np.asarray(Wq, dtype=np.float32)
    Wv = np.asarray(Wv, dtype=np.float32)

    # host marshalling: transpose x so C is leading (partition) dim per batch
    xT = np.ascontiguousarray(x.transpose(0, 2, 1))  # [B, C, T]
    Wkq = np.ascontiguousarray(
        np.concatenate([Wk * np.float32(SCALE), Wq], axis=1)
    )  # [C, 128]

    nc = _get_program()
    in_maps = [
        {"xT": xT[c * BPC : (c + 1) * BPC], "Wkq": Wkq, "Wv": Wv}
        for c in range(N_CORES)
    ]
    res = run_bass_kernel_spmd(
        nc,
        in_maps,
        core_ids=list(range(N_CORES)),
        trace=_trace,
        **(_trace_kwargs or {}),
    )
    outp = np.concatenate(
        [res.results[c]["out"] for c in range(N_CORES)], axis=0
    )
    if _trace:
        _CACHE["last_results"] = res
    return outp


# revision 7
# speedup vs baseline: 1.2612x; 1.2612x over previous
"""TRN2 Bass kernel for single-head causal attention (nn_Head).

Reference math (note k/q role swap vs standard attention):
    k = x @ Wk; q = x @ Wq; v = x @ Wv          # [B, T, H]
    S = (k @ q^T) * C**-0.5                     # [B, T, T], rows = k positions
    S = where(tril, S, -inf); P = softmax(S, axis=-1)
    out = P @ v                                 # [B, T, H]

Strategy (8 NeuronCores, data-parallel over batch, 4 batches/core):
  - Host marshalling: upload x pre-transposed (xT [b, C, T]) so the
    contraction dim C lands on SBUF partitions; fold the 1/sqrt(C) scale
    into Wk; concatenate Wk|Wq so one matmul computes both projections.
  - All matmuls in float32r (TF32-class, full PE rate at free dim >= 256).
  - S^T tiles [b_part, a_free] (b = softmax dim): no max subtraction
    needed (|S| <= ~1.5 by construction), so P = exp(S) and the
    denominator is folded into the PV matmul via a ones-column on V.
  - PV in transposed form: out'^T[h', a] = sum_b V'[b, h'] P^T[b, a]
    accumulated over b-tiles in PSUM; final PE transpose + DVE divide.
  - Batch-level software pipelining: batch b+1's projection matmuls are
    interleaved into batch b's attention phase so TensorE always has
    dense independent work (keeps the HAM clock gate at K=8/8).
"""

import sys

sys.path.insert(0, "/opt/trn_rl_repo")

import numpy as np

import concourse.bass as bass  # noqa: F401
import concourse.mybir as mybir
import concourse.tile as tile
from concourse import bacc
from concourse.bass_utils import run_bass_kernel_spmd
from concourse.masks import make_identity

B, T, C, H = 32, 2048, 1024, 64
N_CORES = 8
BPC = B // N_CORES  # batches per core
SCALE = C**-0.5

F32 = mybir.dt.float32
F32R = mybir.dt.float32r
BF16 = mybir.dt.bfloat16

TCH = 512  # i-chunk width (PSUM bank = 512 fp32)
NTC = T // TCH  # 4 i-chunks
NJT = T // 128  # 16 j-tiles
NCC = C // 128  # 8 contraction chunks
PV_LAG = 3  # S^T(jt) to PV(jt) interleave distance


def build_program():
    nc = bacc.Bacc(
        "TRN2", target_bir_lowering=False, debug=False, num_devices=N_CORES
    )
    xT = nc.dram_tensor("xT", [BPC, C, T], BF16, kind="ExternalInput").ap()
    Wkq = nc.dram_tensor("Wkq", [C, 128], BF16, kind="ExternalInput").ap()
    Wv = nc.dram_tensor("Wv", [C, H], BF16, kind="ExternalInput").ap()
    out = nc.dram_tensor("out", [BPC, T, H], F32, kind="ExternalOutput").ap()

    with tile.TileContext(nc) as tc:
        with (
            tc.tile_pool(name="const", bufs=1) as const,
            tc.tile_pool(name="xtp", bufs=11) as xtp,
            tc.tile_pool(name="ep", bufs=2) as ep,
            tc.tile_pool(name="ptp", bufs=6) as ptp,
            tc.tile_pool(name="psA", bufs=2, space="PSUM") as psA,
            tc.tile_pool(name="psS", bufs=3, space="PSUM") as psS,
            tc.tile_pool(name="psO", bufs=2, space="PSUM") as psO,
            tc.tile_pool(name="psT", bufs=1, space="PSUM") as psT,
        ):
            # --- constants ---
            wkq_sb = const.tile([128, NCC, 128], BF16)
            nc.sync.dma_start(
                out=wkq_sb, in_=Wkq.rearrange("(cc p) h -> p cc h", p=128)
            )
            wv_sb = const.tile([128, NCC, H], BF16)
            nc.sync.dma_start(
                out=wv_sb, in_=Wv.rearrange("(cc p) h -> p cc h", p=128)
            )
            ident = const.tile([128, 128], F32)
            make_identity(nc, ident)
            # causal tile mask: trimask[j, i] = 1.0 if i >= j else 0.0
            # (built in fp32; DVE copy rounds into the fp32r tile)
            trimask_f = const.tile([128, 128], F32)
            nc.vector.memset(trimask_f, 1.0)
            nc.gpsimd.affine_select(
                out=trimask_f,
                in_=trimask_f,
                compare_op=mybir.AluOpType.is_ge,
                fill=0.0,
                base=0,
                pattern=[[1, 128]],
                channel_multiplier=-1,
            )
            trimask = const.tile([128, 128], BF16)
            nc.vector.tensor_copy(trimask, trimask_f)
            ones_f = const.tile([128, 1], F32)
            nc.vector.memset(ones_f, 1.0)
            ident16 = const.tile([128, 128], BF16)
            nc.vector.tensor_copy(ident16, ident)

            state = {}

            def emit_loads(b):
                xts = []
                for cc in range(NCC):
                    xt = xtp.tile([128, T], BF16, name=f"xt_{b}_{cc}", tag="xt")
                    # alternate HWDGE queues (sync/scalar) for 2-way overlap
                    eng = nc.sync if cc % 2 == 0 else nc.scalar
                    eng.dma_start(out=xt, in_=xT[b, cc * 128 : (cc + 1) * 128, :])
                    xts.append(xt)
                state[b] = {"xts": xts}

            def emit_kq_chunk(b, t_):
                st = state[b]
                if "kqT" not in st:
                    st["kqT"] = ep.tile([128, T], BF16, name=f"kqT_{b}", tag="kqT")
                    st["qT"] = ep.tile([64, T], BF16, name=f"qT_{b}", tag="qT")
                sl = slice(t_ * TCH, (t_ + 1) * TCH)
                kq_ps = psA.tile([128, TCH], F32, name=f"kqps_{b}_{t_}", tag="proj")
                for cc in range(NCC):
                    nc.tensor.matmul(
                        kq_ps,
                        wkq_sb[:, cc, :],
                        st["xts"][cc][:, sl],
                        start=(cc == 0),
                        stop=(cc == NCC - 1),
                    )
                nc.vector.tensor_copy(st["kqT"][:, sl], kq_ps)
                # shift q^T half down to partitions 0-63 (SWDGE: keep the
                # sync/scalar HWDGE queues free for bulk loads)
                nc.gpsimd.dma_start(out=st["qT"][:, sl], in_=st["kqT"][64:128, sl])

            def emit_v_chunk(b, t_):
                st = state[b]
                if "vT" not in st:
                    st["vT"] = ep.tile([64, T], BF16, name=f"vT_{b}", tag="vT")
                sl = slice(t_ * TCH, (t_ + 1) * TCH)
                v_ps = psA.tile([64, TCH], F32, name=f"vps_{b}_{t_}", tag="proj")
                for cc in range(NCC):
                    nc.tensor.matmul(
                        v_ps,
                        wv_sb[:, cc, :],
                        st["xts"][cc][:, sl],
                        start=(cc == 0),
                        stop=(cc == NCC - 1),
                    )
                nc.vector.tensor_copy(st["vT"][:, sl], v_ps)

            def emit_vtrans(b, jt):
                st = state[b]
                if "vp" not in st:
                    st["vp"] = ep.tile(
                        [128, NJT, H + 1], BF16, name=f"vp_{b}", tag="vp"
                    )
                tv_ps = psT.tile([128, H + 1], BF16, name=f"tvps_{b}_{jt}", tag="tps")
                nc.tensor.transpose(
                    tv_ps[:, 0:H],
                    st["vT"][:, jt * 128 : (jt + 1) * 128],
                    ident16[0:64, 0:64],
                )
                nc.vector.tensor_copy(st["vp"][:, jt, 0:H], tv_ps[:, 0:H])
                nc.vector.tensor_copy(st["vp"][:, jt, H : H + 1], ones_f)

            def proj_units(b):
                """Work units of batch b's projection phase, for interleaving."""
                yield lambda: emit_kq_chunk(b, 0)
                yield lambda: emit_kq_chunk(b, 1)
                yield lambda: emit_kq_chunk(b, 2)
                yield lambda: emit_kq_chunk(b, 3)
                yield lambda: emit_v_chunk(b, 0)
                yield lambda: emit_v_chunk(b, 1)
                yield lambda: emit_v_chunk(b, 2)
                yield lambda: emit_v_chunk(b, 3)
                for jt0 in range(0, NJT, 4):
                    def vt_group(jt0=jt0):
                        for jt in range(jt0, jt0 + 4):
                            emit_vtrans(b, jt)
                    yield vt_group

            def emit_attention_ci(b, ci):
                st = state[b]
                kqT, qT, vp = st["kqT"], st["qT"], st["vp"]
                jt_max = 4 * ci + 3
                pts = []

                def pv_step(jt):
                    pt, i0, w = pts[jt]
                    nc.tensor.matmul(
                        o_ps[:, i0 - ci * TCH : TCH],
                        vp[:, jt, :],
                        pt[:, 0:w],
                        start=(jt == 0),
                        stop=(jt == jt_max),
                    )

                o_ps = psO.tile([H + 1, TCH], F32, name=f"ops_{b}_{ci}", tag="ops")
                for jt in range(jt_max + 1):
                    i0 = max(ci * TCH, jt * 128)
                    w = (ci + 1) * TCH - i0
                    s_ps = psS.tile(
                        [128, TCH], F32, name=f"sps_{b}_{ci}_{jt}", tag="sps"
                    )
                    # s_ps[b_local, a] = sum_h q[jt*128+b_local, h] k[a, h]
                    nc.tensor.matmul(
                        s_ps[:, 0:w],
                        qT[:, jt * 128 : (jt + 1) * 128],
                        kqT[0:64, i0 : i0 + w],
                        start=True,
                        stop=True,
                    )
                    pt = ptp.tile([128, TCH], BF16, name=f"pt_{b}_{ci}_{jt}", tag="pt")
                    nc.scalar.activation(
                        pt[:, 0:w], s_ps[:, 0:w], mybir.ActivationFunctionType.Exp
                    )
                    if jt >= 4 * ci:
                        nc.vector.tensor_mul(pt[:, 0:128], pt[:, 0:128], trimask)
                    pts.append((pt, i0, w))
                    if jt >= PV_LAG:
                        pv_step(jt - PV_LAG)
                for jt in range(max(0, jt_max + 1 - PV_LAG), jt_max + 1):
                    pv_step(jt)

                # --- epilogue: transpose out'^T, divide by denominator ---
                o_sb = ep.tile([H + 1, TCH], F32, name=f"osb_{b}_{ci}", tag="osb")
                nc.vector.tensor_copy(o_sb, o_ps)
                out_sb = ep.tile([128, 4, H], F32, name=f"outsb_{b}_{ci}", tag="outsb")
                for it in range(4):
                    to_ps = psT.tile(
                        [128, H + 1], F32, name=f"tops_{b}_{ci}_{it}", tag="tps"
                    )
                    nc.tensor.transpose(
                        to_ps,
                        o_sb[:, it * 128 : (it + 1) * 128],
                        ident[0 : H + 1, 0 : H + 1],
                    )
                    r_sb = ep.tile([128, 1], F32, name=f"rsb_{b}_{ci}_{it}", tag="rsb")
                    nc.vector.reciprocal(r_sb, to_ps[:, H : H + 1])
                    nc.vector.tensor_scalar_mul(out_sb[:, it, :], to_ps[:, 0:H], r_sb)
                nc.gpsimd.dma_start(
                    out=out[b, ci * TCH : (ci + 1) * TCH, :].rearrange(
                        "(it p) h -> p it h", p=128
                    ),
                    in_=out_sb,
                )

            # --- schedule: batch-level software pipeline ---
            emit_loads(0)
            for u in proj_units(0):
                u()
            # filler distribution: after each attention ci-group of batch b,
            # emit some of batch b+1's projection work
            fill_after_ci = [2, 3, 3, 4]  # sums to 12 units
            for b in range(BPC):
                if b + 1 < BPC:
                    emit_loads(b + 1)
                    filler = list(proj_units(b + 1))
                else:
                    filler = []
                fi = 0
                for ci in range(NTC):
                    emit_attention_ci(b, ci)
                    for _ in range(fill_after_ci[ci]):
                        if fi < len(filler):
                            filler[fi]()
                            fi += 1
                while fi < len(filler):
                    filler[fi]()
                    fi += 1

    nc.compile()
    return nc


_CACHE = {}


def _get_program():
    if "nc" not in _CACHE:
        _CACHE["nc"] = build_program()
    return _CACHE["nc"]


def kernel(x, Wk, Wq, Wv, _trace=False, _trace_kwargs=None):
    x = np.asarray(x, dtype=np.float32)
    Wk = np.asarray(Wk, dtype=np.float32)
    Wq = np.asarray(Wq, dtype=np.float32)
    Wv = np.asarray(Wv, dtype=np.float32)

    import ml_dtypes

    # host marshalling: transpose x so C is leading (partition) dim per batch
    bf = ml_dtypes.bfloat16
    xT = np.ascontiguousarray(x.transpose(0, 2, 1)).astype(bf)  # [B, C, T]
    Wkq = np.ascontiguousarray(
        np.concatenate([Wk * np.float32(SCALE), Wq], axis=1)
    ).astype(bf)  # [C, 128]
    Wv = Wv.astype(bf)

    nc = _get_program()
    in_maps = [
        {"xT": xT[c * BPC : (c + 1) * BPC], "Wkq": Wkq, "Wv": Wv}
        for c in range(N_CORES)
    ]
    res = run_bass_kernel_spmd(
        nc,
        in_maps,
        core_ids=list(range(N_CORES)),
        trace=_trace,
        **(_trace_kwargs or {}),
    )
    outp = np.concatenate(
        [res.results[c]["out"] for c in range(N_CORES)], axis=0
    )
    if _trace:
        _CACHE["last_results"] = res
    return outp
